# revision 18
# baseline (speedup 1.0000x reference)
"""PixelPrototypeDistanceLoss on 8 Trainium2 NeuronCores.

Math: for each pixel p with label lb_p != 19:
    logit_p = emb_pixel_p . segment_queue[lb_p]
    loss = mean((1 - logit_p)^2)  over valid pixels

Trick: with onehot[c,p] = (lb_p == c) for c in [0,19), ignored pixels match
nothing, so
    sum_p valid*(1-logit)^2 = count - 2*S1 + S2
with count = sum(onehot), S1 = sum(sim*onehot), S2 = sum(sim^2*onehot),
all plain full reductions over the [C, N] similarity map -- no gather.

Sharding: batch dim across the 8 cores (one image each).  Per core:
  sim tiles [19, 512] computed as QT.T @ X with X = emb[b] reshaped [256, N]
  (already channels-first, no transpose needed).  Four pixel-blocks stacked
  at partition offsets 0/32/64/96 (PE tile_position) so the DVE sees
  [128, C_g] blocks; the four quadrant matmuls execute concurrently on the
  PE which is what keeps it at DMA pace (measured ~0.3 ns/moving-col
  aggregate vs ~1.2 serial).  QT is zero-padded to 32 columns so every PSUM
  row is written.  scalar_tensor_tensor fuses onehot*sim with the S1
  row-sum; ScalarE activation(Square) accumulates S2 (last group's S2 done
  as a DVE tensor_tensor instead -- keeps the tail on one engine).
  Onehot ships from host as u8 (on-device onehot via PE broadcast + Relu
  was tried: it pushes PE and ScalarE above the DMA roofline, net loss).
Warm-up: ~24 matmuls on a zeroed scratch right after the engine barrier --
  the PE HAM clock-gates after >3.4us idle and a cold PE runs matmuls at
  half clock; warm matmuls during the DMA window buy full clock when the
  real tiles land.
Tail: one accumulator tile [128, 13] (count | S1 x6 | S2 x6) DMAed out
  directly; host does the final partition sum.  No PE reduce, no copy, no
  cross-engine hops after the last group's DVE ops.
Pipelining: emb cast to fp8-e4m3 on host (memory-bound); all tiles
  resident; DMAs issued upfront on one HWDGE queue; small x tiles at the
  end shorten the post-stream drain.
Host: sums the tiny per-core partial accumulators in f64.
"""

import numpy as np
import ml_dtypes

import concourse.bacc as bacc
import concourse.mybir as mybir
from concourse.tile import TileContext
from concourse import bass_utils

# Problem dims (hardcoded per harness contract).
B, D, H, W, C = 8, 256, 128, 128, 19
NPX = H * W          # 16384 pixels per core (one batch image)
NCORES = 8
IGNORE = 19.0

CP = 32              # padded class count (PE tile_position granularity)
F = 512              # max matmul out free dim (one PSUM bank of f32)
# x DMA tiles (pixel counts): small first tile -> early PE start; small
# tail tiles -> short post-stream drain
XTILES = [2048, 4096, 4096, 4096, 1536, 512]
assert sum(XTILES) == NPX
NG = len(XTILES)
CGS = [n // 4 for n in XTILES]          # onehot/psum cols per group
OFFS = np.concatenate([[0], np.cumsum(CGS)]).tolist()
LBB_COLS = NPX // 4                      # 4096
NWARM = 24

EMB_DT = mybir.dt.float8e4
EMB_NP = ml_dtypes.float8_e4m3

META_COLS = 2 * CP + 4 + 128            # qt fp8 | ones f32 | labels u8

_CACHE = {}


def _build():
    if "nc" in _CACHE:
        return _CACHE["nc"]
    nc = bacc.Bacc(
        "TRN2",
        target_bir_lowering=False,
        debug=False,
        enable_asserts=False,
    )
    # x packed on host as [128, 2*NPX]: group g's block at cols
    # [2*base_g, 2*base_g + 2*n); within a block col k*n+j = emb k-half
    x_t = nc.dram_tensor("x", [128, 2 * NPX], EMB_DT, kind="ExternalInput")
    # meta: cols 0:64 = qt fp8 bytes (col 32k+c = QT[128k+p, c]),
    # cols 64:68 = 1.0f (unused spare), cols 68:196 = labels as u8
    meta_t = nc.dram_tensor("meta", [128, META_COLS], mybir.dt.uint8,
                            kind="ExternalInput")
    # onehot: [128, NPX/4] u8, col off_g+j partition 32s+c =
    # (lb[base_g + s*cg + j] == c)
    lbb_t = nc.dram_tensor("lbb", [128, LBB_COLS], mybir.dt.uint8,
                           kind="ExternalInput")
    out_t = nc.dram_tensor("out", [128, 1 + 2 * NG], mybir.dt.float32,
                           kind="ExternalOutput")

    x = x_t.ap()
    meta = meta_t.ap()
    lbb = lbb_t.ap()
    out = out_t.ap()

    AO = mybir.AluOpType

    with TileContext(nc) as tc:
        with (
            tc.tile_pool(name="const", bufs=1) as cpool,
            tc.tile_pool(name="xp", bufs=1) as xpool,
            tc.tile_pool(name="scr", bufs=3) as spool,
            tc.tile_pool(name="acc", bufs=1) as apool,
            tc.tile_pool(name="psA", bufs=2, space="PSUM") as psa,
            tc.tile_pool(name="psW", bufs=1, space="PSUM") as psw,
        ):
            # all input tiles are resident; issue every DMA upfront on ONE
            # HWDGE queue (two queues contend for a shared cap and starve
            # each other; a single queue sustains ~430 GB/s here)
            metat = cpool.tile([128, META_COLS], mybir.dt.uint8)
            nc.sync.dma_start(metat[:, :], meta[:, :])
            lbbt = cpool.tile([128, LBB_COLS], mybir.dt.uint8)
            nc.sync.dma_start(lbbt[:, :], lbb[:, :])
            xt = {}
            base = 0
            for g, n in enumerate(XTILES):
                t = xpool.tile([128, 2 * n], EMB_DT, tag=f"xg{g}")
                nc.sync.dma_start(t[:, :], x[:, 2 * base:2 * base + 2 * n])
                xt[g] = t
                base += n

            qt_sb = metat[:, 0:2 * CP].bitcast(EMB_DT)
            lb_sb = metat[:, 2 * CP + 4:META_COLS]

            acc = apool.tile([128, 1 + 2 * NG], mybir.dt.float32)
            junk = apool.tile([128, 128], mybir.dt.float32)
            t2 = apool.tile([128, max(CGS)], mybir.dt.float32)
            t2v = apool.tile([128, CGS[-1]], mybir.dt.float32)

            # PE warm-up: zeroed fp8 scratch, quadrant-rotating matmuls.
            # Nothing reads psW; the only goal is keeping the PE HAM
            # active from the barrier until the first x tile lands.
            warm = apool.tile([128, F], EMB_DT)
            nc.gpsimd.memset(warm[:, :], 0)
            psW = psw.tile([128, F], mybir.dt.float32)
            for i in range(NWARM):
                s = i % 4
                nc.tensor.matmul(out=psW[CP * s:CP * (s + 1), :],
                                 lhsT=warm[:, 0:CP], rhs=warm[:, :],
                                 start=True, stop=True,
                                 tile_position=(0, CP * s))

            # count of valid pixels (per partition; host sums).
            nc.vector.tensor_scalar(junk[:, :], lb_sb[:, :], IGNORE, None,
                                    AO.not_equal, AO.add,
                                    accum_out=acc[:, 0:1])

            for g, n in enumerate(XTILES):
                cg = CGS[g]
                ps = psa.tile([128, cg], mybir.dt.float32, tag="psA")
                for s in range(4):
                    for m in range(0, cg, F):
                        fb = min(F, cg - m)
                        for k in range(2):
                            col = k * n + s * cg + m
                            nc.tensor.matmul(
                                out=ps[CP * s:CP * (s + 1), m:m + fb],
                                lhsT=qt_sb[:, k * CP:(k + 1) * CP],
                                rhs=xt[g][:, col:col + fb],
                                start=(k == 0), stop=(k == 1),
                                tile_position=(0, CP * s))

                t1 = spool.tile([128, cg], mybir.dt.float32, tag="t1")
                # t1 = onehot * sim ; acc[:, 1+g] = row-sum(t1)
                nc.vector.scalar_tensor_tensor(
                    out=t1[:, :], in0=lbbt[:, OFFS[g]:OFFS[g] + cg],
                    scalar=1.0, in1=ps[:, :], op0=AO.mult, op1=AO.mult,
                    accum_out=acc[:, 1 + g:2 + g])
                if g < NG - 1:
                    # t2 = t1^2 = onehot*sim^2 ; acc[:, 1+NG+g] = row-sum
                    # on the otherwise-idle scalar engine
                    nc.scalar.activation(
                        t2[:, 0:cg], t1[:, :],
                        mybir.ActivationFunctionType.Square,
                        accum_out=acc[:, 1 + NG + g:2 + NG + g])
                else:
                    # last group stays on the DVE: no cross-engine hop in
                    # the tail (t1 * sim == onehot * sim^2)
                    nc.vector.scalar_tensor_tensor(
                        out=t2v[:, :], in0=t1[:, :], scalar=1.0,
                        in1=ps[:, :], op0=AO.mult, op1=AO.mult,
                        accum_out=acc[:, 1 + NG + g:2 + NG + g])

            # ship the raw per-partition accumulators; host reduces
            nc.sync.dma_start(out[:, :], acc[:, :])

    nc.compile()
    _CACHE["nc"] = nc
    return nc


def _prep_in_maps(emb, lb, segment_queue):
    emb = np.asarray(emb)
    lb = np.asarray(lb)
    q = np.asarray(segment_queue, dtype=np.float32)

    qt = np.zeros((D, CP), np.float32)
    qt[:, :C] = q.T
    # pack [2,128,CP] -> [128, 2*CP]: col 32k+c = QT[128k+p, c]
    qt = np.ascontiguousarray(
        qt.reshape(2, 128, CP).transpose(1, 0, 2).reshape(128, 2 * CP)
        .astype(EMB_NP))

    cls_pat = np.where(np.arange(CP) < C, np.arange(CP), -1)  # [32]

    in_maps = []
    for b in range(B):
        x8 = emb[b].reshape(2, 128, NPX).astype(EMB_NP)
        # pack per DMA tile: xb[p, 2*base + k*n + j] = x8[k, p, base + j]
        xb = np.empty((128, 2 * NPX), EMB_NP)
        base = 0
        for n in XTILES:
            blk = x8[:, :, base:base + n]            # [2, 128, n]
            xb[:, 2 * base:2 * base + 2 * n] = (
                blk.transpose(1, 0, 2).reshape(128, 2 * n))
            base += n
        lbf = lb[b].reshape(-1).astype(np.float32)
        # onehot[32*s + c, off_g + j] = (lb[base_g + s*C_g + j] == c)
        segs = []
        base = 0
        for n in XTILES:
            cg = n // 4
            seg = lbf[base:base + n].reshape(4, 1, cg)
            segs.append((seg == cls_pat[None, :, None]).reshape(128, cg))
            base += n
        lbb = np.concatenate(segs, axis=1).astype(np.uint8)

        meta = np.empty((128, META_COLS), np.uint8)
        meta[:, :2 * CP] = qt.view(np.uint8)
        meta[:, 2 * CP:2 * CP + 4] = (
            np.ones((128, 1), np.float32).view(np.uint8))
        meta[:, 2 * CP + 4:] = lbf.reshape(128, 128).astype(np.uint8)

        in_maps.append({
            "x": xb,
            "meta": np.ascontiguousarray(meta),
            "lbb": np.ascontiguousarray(lbb),
        })
    return in_maps


def _reduce_outputs(results):
    cnt = 0.0
    s1 = 0.0
    s2 = 0.0
    for r in results:
        o = np.asarray(r["out"], dtype=np.float64)
        cnt += o[:, 0].sum()
        s1 += o[:, 1:1 + NG].sum()
        s2 += o[:, 1 + NG:1 + 2 * NG].sum()
    num = cnt - 2.0 * s1 + s2
    return np.float32(num / cnt)


def run_on_cores(inputs, **kwargs):
    """Run the bass kernel on cores 0-7; returns (loss, BassKernelResults).

    The device occasionally reports a transient NRT_EXEC_UNIT_UNRECOVERABLE
    on a run that succeeds on immediate retry; retry a couple of times.
    """
    nc = _build()
    in_maps = _prep_in_maps(**inputs)
    last_err = None
    for _ in range(3):
        try:
            res = bass_utils.run_bass_kernel_spmd(
                nc, in_maps, core_ids=list(range(NCORES)), **kwargs)
            return _reduce_outputs(res.results), res
        except Exception as e:  # transient device wedge -> retry
            last_err = e
    raise last_err


def kernel(emb, lb, segment_queue):
    loss, _ = run_on_cores({"emb": emb, "lb": lb, "segment_queue": segment_queue})
    return loss


# revision 19
# speedup vs baseline: 1.1310x; 1.1310x over previous
"""PixelPrototypeDistanceLoss on 8 Trainium2 NeuronCores.

Math: for each pixel p with label lb_p != 19:
    logit_p = emb_pixel_p . segment_queue[lb_p]
    loss = mean((1 - logit_p)^2)  over valid pixels

Trick: with onehot[c,p] = (lb_p == c) for c in [0,19), ignored pixels match
nothing, so
    sum_p valid*(1-logit)^2 = count - 2*S1 + S2
with count = sum(onehot), S1 = sum(sim*onehot), S2 = sum(sim^2*onehot),
all plain full reductions over the [C, N] similarity map -- no gather.

Sharding: batch dim across the 8 cores (one image each).  Per core:
  sim tiles [19, 512] computed as QT.T @ X with X = emb[b] reshaped [256, N]
  (already channels-first, no transpose needed).  Four pixel-blocks stacked
  at partition offsets 0/32/64/96 (PE tile_position) so the DVE sees
  [128, C_g] blocks; the four quadrant matmuls execute concurrently on the
  PE which is what keeps it at DMA pace (measured ~0.3 ns/moving-col
  aggregate vs ~1.2 serial).  QT is zero-padded to 32 columns so every PSUM
  row is written.  scalar_tensor_tensor fuses onehot*sim with the S1
  row-sum; ScalarE activation(Square) accumulates S2 (last group's S2 done
  as a DVE tensor_tensor instead -- keeps the tail on one engine).
  Onehot ships from host as u8 (on-device onehot via PE broadcast + Relu
  was tried: it pushes PE and ScalarE above the DMA roofline, net loss).
Warm-up: ~24 matmuls on a zeroed scratch right after the engine barrier --
  the PE HAM clock-gates after >3.4us idle and a cold PE runs matmuls at
  half clock; warm matmuls during the DMA window buy full clock when the
  real tiles land.
Tail: one accumulator tile [128, 13] (count | S1 x6 | S2 x6) DMAed out
  directly; host does the final partition sum.  No PE reduce, no copy, no
  cross-engine hops after the last group's DVE ops.
Pipelining: emb cast to fp8-e4m3 on host (memory-bound); all tiles
  resident; DMAs issued upfront on one HWDGE queue; small x tiles at the
  end shorten the post-stream drain.
Host: sums the tiny per-core partial accumulators in f64.
"""

import numpy as np
import ml_dtypes

import concourse.bacc as bacc
import concourse.mybir as mybir
from concourse.tile import TileContext
from concourse import bass_utils

# Problem dims (hardcoded per harness contract).
B, D, H, W, C = 8, 256, 128, 128, 19
NPX = H * W          # 16384 pixels per core (one batch image)
NCORES = 8
IGNORE = 19.0

CP = 32              # padded class count (PE tile_position granularity)
F = 512              # max matmul out free dim (one PSUM bank of f32)
# x DMA tiles (pixel counts): small first tile -> early PE start; small
# tail tiles -> short post-stream drain
XTILES = [2048, 4096, 4096, 4096, 1536, 512]
assert sum(XTILES) == NPX
NG = len(XTILES)
CGS = [n // 4 for n in XTILES]          # onehot/psum cols per group
OFFS = np.concatenate([[0], np.cumsum(CGS)]).tolist()
LBB_COLS = NPX // 4                      # 4096
NWARM = 24

EMB_DT = mybir.dt.float8e4
EMB_NP = ml_dtypes.float8_e4m3

META_COLS = 2 * CP + 4 + 128            # qt fp8 | ones f32 | labels u8

_CACHE = {}


def _build():
    if "nc" in _CACHE:
        return _CACHE["nc"]
    nc = bacc.Bacc(
        "TRN2",
        target_bir_lowering=False,
        debug=False,
        enable_asserts=False,
    )
    # x packed on host as [128, 2*NPX]: group g's block at cols
    # [2*base_g, 2*base_g + 2*n); within a block col k*n+j = emb k-half
    x_t = nc.dram_tensor("x", [128, 2 * NPX], EMB_DT, kind="ExternalInput")
    # meta: cols 0:64 = qt fp8 bytes (col 32k+c = QT[128k+p, c]),
    # cols 64:68 = 1.0f (unused spare), cols 68:196 = labels as u8
    meta_t = nc.dram_tensor("meta", [128, META_COLS], mybir.dt.uint8,
                            kind="ExternalInput")
    # onehot: [128, NPX/4] u8, col off_g+j partition 32s+c =
    # (lb[base_g + s*cg + j] == c)
    lbb_t = nc.dram_tensor("lbb", [128, LBB_COLS], mybir.dt.uint8,
                           kind="ExternalInput")
    out_t = nc.dram_tensor("out", [128, 1 + 2 * NG], mybir.dt.float32,
                           kind="ExternalOutput")

    x = x_t.ap()
    meta = meta_t.ap()
    lbb = lbb_t.ap()
    out = out_t.ap()

    AO = mybir.AluOpType

    with TileContext(nc) as tc:
        with (
            tc.tile_pool(name="const", bufs=1) as cpool,
            tc.tile_pool(name="xp", bufs=1) as xpool,
            tc.tile_pool(name="scr", bufs=3) as spool,
            tc.tile_pool(name="acc", bufs=1) as apool,
            tc.tile_pool(name="psA", bufs=3, space="PSUM") as psa,
        ):
            # all input tiles are resident; issue every DMA upfront on ONE
            # HWDGE queue (two queues contend for a shared cap and starve
            # each other; a single queue sustains ~430 GB/s here)
            metat = cpool.tile([128, META_COLS], mybir.dt.uint8)
            nc.sync.dma_start(metat[:, :], meta[:, :])
            lbbt = cpool.tile([128, LBB_COLS], mybir.dt.uint8)
            xt = {}
            base = 0
            for g, n in enumerate(XTILES):
                t = xpool.tile([128, 2 * n], EMB_DT, tag=f"xg{g}")
                nc.sync.dma_start(t[:, :], x[:, 2 * base:2 * base + 2 * n])
                xt[g] = t
                base += n
                if g == 0:
                    # onehot lands after x0: PE needs only qt+x0 to start;
                    # the DVE (which needs lbb) runs well behind the PE
                    nc.sync.dma_start(lbbt[:, :], lbb[:, :])

            qt_sb = metat[:, 0:2 * CP].bitcast(EMB_DT)
            lb_sb = metat[:, 2 * CP + 4:META_COLS]

            acc = apool.tile([128, 1 + 2 * NG], mybir.dt.float32)
            junk = apool.tile([128, 128], mybir.dt.float32)
            t2 = apool.tile([128, max(CGS)], mybir.dt.float32)
            t2v = apool.tile([128, max(CGS[-2:])], mybir.dt.float32)

            # count of valid pixels (per partition; host sums).
            nc.vector.tensor_scalar(junk[:, :], lb_sb[:, :], IGNORE, None,
                                    AO.not_equal, AO.add,
                                    accum_out=acc[:, 0:1])

            for g, n in enumerate(XTILES):
                cg = CGS[g]
                ps = psa.tile([128, cg], mybir.dt.float32, tag="psA")
                for s in range(4):
                    for m in range(0, cg, F):
                        fb = min(F, cg - m)
                        for k in range(2):
                            col = k * n + s * cg + m
                            nc.tensor.matmul(
                                out=ps[CP * s:CP * (s + 1), m:m + fb],
                                lhsT=qt_sb[:, k * CP:(k + 1) * CP],
                                rhs=xt[g][:, col:col + fb],
                                start=(k == 0), stop=(k == 1),
                                tile_position=(0, CP * s))

                t1 = spool.tile([128, cg], mybir.dt.float32, tag="t1")
                # t1 = onehot * sim ; acc[:, 1+g] = row-sum(t1)
                nc.vector.scalar_tensor_tensor(
                    out=t1[:, :], in0=lbbt[:, OFFS[g]:OFFS[g] + cg],
                    scalar=1.0, in1=ps[:, :], op0=AO.mult, op1=AO.mult,
                    accum_out=acc[:, 1 + g:2 + g])
                if g < NG - 2:
                    # t2 = t1^2 = onehot*sim^2 ; acc[:, 1+NG+g] = row-sum
                    # on the otherwise-idle scalar engine
                    nc.scalar.activation(
                        t2[:, 0:cg], t1[:, :],
                        mybir.ActivationFunctionType.Square,
                        accum_out=acc[:, 1 + NG + g:2 + NG + g])
                else:
                    # last group stays on the DVE: no cross-engine hop in
                    # the tail (t1 * sim == onehot * sim^2)
                    nc.vector.scalar_tensor_tensor(
                        out=t2v[:, 0:cg], in0=t1[:, :], scalar=1.0,
                        in1=ps[:, :], op0=AO.mult, op1=AO.mult,
                        accum_out=acc[:, 1 + NG + g:2 + NG + g])

            # ship the raw per-partition accumulators; host reduces
            nc.sync.dma_start(out[:, :], acc[:, :])

    nc.compile()
    _CACHE["nc"] = nc
    return nc


def _prep_in_maps(emb, lb, segment_queue):
    emb = np.asarray(emb)
    lb = np.asarray(lb)
    q = np.asarray(segment_queue, dtype=np.float32)

    qt = np.zeros((D, CP), np.float32)
    qt[:, :C] = q.T
    # pack [2,128,CP] -> [128, 2*CP]: col 32k+c = QT[128k+p, c]
    qt = np.ascontiguousarray(
        qt.reshape(2, 128, CP).transpose(1, 0, 2).reshape(128, 2 * CP)
        .astype(EMB_NP))

    cls_pat = np.where(np.arange(CP) < C, np.arange(CP), -1)  # [32]

    in_maps = []
    for b in range(B):
        x8 = emb[b].reshape(2, 128, NPX).astype(EMB_NP)
        # pack per DMA tile: xb[p, 2*base + k*n + j] = x8[k, p, base + j]
        xb = np.empty((128, 2 * NPX), EMB_NP)
        base = 0
        for n in XTILES:
            blk = x8[:, :, base:base + n]            # [2, 128, n]
            xb[:, 2 * base:2 * base + 2 * n] = (
                blk.transpose(1, 0, 2).reshape(128, 2 * n))
            base += n
        lbf = lb[b].reshape(-1).astype(np.float32)
        # onehot[32*s + c, off_g + j] = (lb[base_g + s*C_g + j] == c)
        segs = []
        base = 0
        for n in XTILES:
            cg = n // 4
            seg = lbf[base:base + n].reshape(4, 1, cg)
            segs.append((seg == cls_pat[None, :, None]).reshape(128, cg))
            base += n
        lbb = np.concatenate(segs, axis=1).astype(np.uint8)

        meta = np.empty((128, META_COLS), np.uint8)
        meta[:, :2 * CP] = qt.view(np.uint8)
        meta[:, 2 * CP:2 * CP + 4] = (
            np.ones((128, 1), np.float32).view(np.uint8))
        meta[:, 2 * CP + 4:] = lbf.reshape(128, 128).astype(np.uint8)

        in_maps.append({
            "x": xb,
            "meta": np.ascontiguousarray(meta),
            "lbb": np.ascontiguousarray(lbb),
        })
    return in_maps


def _reduce_outputs(results):
    cnt = 0.0
    s1 = 0.0
    s2 = 0.0
    for r in results:
        o = np.asarray(r["out"], dtype=np.float64)
        cnt += o[:, 0].sum()
        s1 += o[:, 1:1 + NG].sum()
        s2 += o[:, 1 + NG:1 + 2 * NG].sum()
    num = cnt - 2.0 * s1 + s2
    return np.float32(num / cnt)


def run_on_cores(inputs, **kwargs):
    """Run the bass kernel on cores 0-7; returns (loss, BassKernelResults).

    The device occasionally reports a transient NRT_EXEC_UNIT_UNRECOVERABLE
    on a run that succeeds on immediate retry; retry a couple of times.
    """
    nc = _build()
    in_maps = _prep_in_maps(**inputs)
    last_err = None
    for _ in range(3):
        try:
            res = bass_utils.run_bass_kernel_spmd(
                nc, in_maps, core_ids=list(range(NCORES)), **kwargs)
            return _reduce_outputs(res.results), res
        except Exception as e:  # transient device wedge -> retry
            last_err = e
    raise last_err


def kernel(emb, lb, segment_queue):
    loss, _ = run_on_cores({"emb": emb, "lb": lb, "segment_queue": segment_queue})
    return loss


# revision 20
# speedup vs baseline: 1.1880x; 1.0504x over previous
"""PixelPrototypeDistanceLoss on 8 Trainium2 NeuronCores.

Math: for each pixel p with label lb_p != 19:
    logit_p = emb_pixel_p . segment_queue[lb_p]
    loss = mean((1 - logit_p)^2)  over valid pixels

Trick: with onehot[c,p] = (lb_p == c) for c in [0,19), ignored pixels match
nothing, so
    sum_p valid*(1-logit)^2 = count - 2*S1 + S2
with count = sum(onehot), S1 = sum(sim*onehot), S2 = sum(sim^2*onehot),
all plain full reductions over the [C, N] similarity map -- no gather.

Sharding: batch dim across the 8 cores (one image each).  Per core:
  sim tiles [19, 512] computed as QT.T @ X with X = emb[b] reshaped [256, N]
  (already channels-first, no transpose needed).  Four pixel-blocks stacked
  at partition offsets 0/32/64/96 (PE tile_position) so the DVE sees
  [128, C_g] blocks; the four quadrant matmuls execute concurrently on the
  PE (measured ~0.3 ns/moving-col aggregate vs ~1.2 for one stream), which
  is what keeps the PE at DMA pace.  QT zero-padded to 32 cols so every
  PSUM row is written.  scalar_tensor_tensor fuses onehot*sim with the S1
  row-sum; ScalarE activation(Square) accumulates S2 except the last two
  groups, whose S2 runs as a second DVE stt (t1*sim) -- ScalarE is still
  busy with earlier squares at that point and a cross-engine hop in the
  tail costs ~0.7us.
Stream layout (one HWDGE queue, issued upfront; boundaries between
  queued transfers are ~free, so many small tiles):
  1. [qt | x tile 0]   -- PE needs only this to start (~10.5us)
  2. [onehot | labels] -- DVE work is never stream-critical
  3+ x tiles 1..9, 2048px each, small at the end so the post-stream
     PE+DVE drain is short.
  Big-row packing throughout: every transfer has >=2KB rows so no
  descriptor-rate penalty (a separate 196B-row meta burned ~1us).
Tail: one accumulator tile [128, 21] (count | S1 x10 | S2 x10) DMAed out
  directly; host does the final partition sum.  No PE reduce, no copy,
  no cross-engine hops after the last group's DVE ops.
Host: emb cast to fp8-e4m3 (memory-bound problem), per-core partial sums
  reduced in f64.
(Tried and rejected: on-device onehot via PE label-broadcast + Relu
  [pushes PE+ScalarE over the DMA roofline], fp8 DoubleRow matmuls [ISA
  allows dst partition 0 only -- kills quadrant stacking], PE warm-up
  matmuls [steals SBUF bandwidth from the DMA stream, delays real work],
  tensor_tensor_reduce [NRT_EXEC_UNIT_UNRECOVERABLE on hw].)
"""

import numpy as np
import ml_dtypes

import concourse.bacc as bacc
import concourse.mybir as mybir
from concourse.tile import TileContext
from concourse import bass_utils

# Problem dims (hardcoded per harness contract).
B, D, H, W, C = 8, 256, 128, 128, 19
NPX = H * W          # 16384 pixels per core (one batch image)
NCORES = 8
IGNORE = 19.0

CP = 32              # padded class count (PE tile_position granularity)
F = 512              # max matmul out free dim (one PSUM bank of f32)
# x tiles (pixel counts): 2048 keeps PE per-group time ~= DMA per-group
# time; small tail tiles shorten the post-stream drain
XTILES = [2048, 2048, 2048, 2048, 2048, 2048, 2048, 1024, 512, 512]
assert sum(XTILES) == NPX
NG = len(XTILES)
CGS = [n // 4 for n in XTILES]          # onehot/psum cols per group
OFFS = np.concatenate([[0], np.cumsum(CGS)]).tolist()
LBB_COLS = NPX // 4                      # 4096
NDVE_S2 = 2                              # trailing groups with S2 on DVE

EMB_DT = mybir.dt.float8e4
EMB_NP = ml_dtypes.float8_e4m3

X0_COLS = 2 * CP + 2 * XTILES[0]        # qt | x tile 0
LBL_COLS = LBB_COLS + 128               # onehot | labels

_CACHE = {}


def _build():
    if "nc" in _CACHE:
        return _CACHE["nc"]
    nc = bacc.Bacc(
        "TRN2",
        target_bir_lowering=False,
        debug=False,
        enable_asserts=False,
    )
    # x0m: cols 0:64 = qt fp8 (col 32k+c = QT[128k+p, c]), then x tile 0
    # packed as [128, 2n] with col k*n+j = emb k-half
    x0m_t = nc.dram_tensor("x0m", [128, X0_COLS], EMB_DT,
                           kind="ExternalInput")
    # xr: remaining x tiles 1.., concatenated [128, 2n] blocks
    XR_COLS = 2 * (NPX - XTILES[0])
    xr_t = nc.dram_tensor("xr", [128, XR_COLS], EMB_DT,
                          kind="ExternalInput")
    # lbl: onehot[32s+c, off_g+j] = (lb[base_g+s*cg+j] == c), then raw
    # labels as u8 for the valid count
    lbl_t = nc.dram_tensor("lbl", [128, LBL_COLS], mybir.dt.uint8,
                           kind="ExternalInput")
    out_t = nc.dram_tensor("out", [128, 1 + 2 * NG], mybir.dt.float32,
                           kind="ExternalOutput")

    x0m = x0m_t.ap()
    xr = xr_t.ap()
    lbl = lbl_t.ap()
    out = out_t.ap()

    AO = mybir.AluOpType

    with TileContext(nc) as tc:
        with (
            tc.tile_pool(name="const", bufs=1) as cpool,
            tc.tile_pool(name="xp", bufs=1) as xpool,
            tc.tile_pool(name="scr", bufs=3) as spool,
            tc.tile_pool(name="acc", bufs=1) as apool,
            tc.tile_pool(name="psA", bufs=3, space="PSUM") as psa,
        ):
            # all tiles resident; all DMAs issued upfront on ONE queue
            x0t = cpool.tile([128, X0_COLS], EMB_DT)
            nc.sync.dma_start(x0t[:, :], x0m[:, :])
            lblt = cpool.tile([128, LBL_COLS], mybir.dt.uint8)
            nc.sync.dma_start(lblt[:, :], lbl[:, :])
            xt = {0: None}
            base = 0
            for g, n in enumerate(XTILES[1:], start=1):
                t = xpool.tile([128, 2 * n], EMB_DT, tag=f"xg{g}")
                nc.sync.dma_start(t[:, :], xr[:, 2 * base:2 * base + 2 * n])
                xt[g] = t
                base += n

            qt_sb = x0t[:, 0:2 * CP]
            lbbt = lblt[:, 0:LBB_COLS]
            lb_sb = lblt[:, LBB_COLS:LBL_COLS]

            acc = apool.tile([128, 1 + 2 * NG], mybir.dt.float32)
            junk = apool.tile([128, 128], mybir.dt.float32)
            t2 = apool.tile([128, max(CGS)], mybir.dt.float32)
            t2v = apool.tile([128, max(CGS[-NDVE_S2:])], mybir.dt.float32)

            # count of valid pixels (per partition; host sums)
            nc.vector.tensor_scalar(junk[:, :], lb_sb[:, :], IGNORE, None,
                                    AO.not_equal, AO.add,
                                    accum_out=acc[:, 0:1])

            for g, n in enumerate(XTILES):
                cg = CGS[g]
                xsrc = x0t if g == 0 else xt[g]
                xoff = 2 * CP if g == 0 else 0
                ps = psa.tile([128, cg], mybir.dt.float32, tag="psA")
                for s in range(4):
                    for m in range(0, cg, F):
                        fb = min(F, cg - m)
                        for k in range(2):
                            col = xoff + k * n + s * cg + m
                            nc.tensor.matmul(
                                out=ps[CP * s:CP * (s + 1), m:m + fb],
                                lhsT=qt_sb[:, k * CP:(k + 1) * CP],
                                rhs=xsrc[:, col:col + fb],
                                start=(k == 0), stop=(k == 1),
                                tile_position=(0, CP * s))

                t1 = spool.tile([128, cg], mybir.dt.float32, tag="t1")
                # t1 = onehot * sim ; acc[:, 1+g] = row-sum(t1)
                nc.vector.scalar_tensor_tensor(
                    out=t1[:, :], in0=lbbt[:, OFFS[g]:OFFS[g] + cg],
                    scalar=1.0, in1=ps[:, :], op0=AO.mult, op1=AO.mult,
                    accum_out=acc[:, 1 + g:2 + g])
                if g < NG - NDVE_S2:
                    # t2 = t1^2 = onehot*sim^2 ; acc[:, 1+NG+g] = row-sum
                    # on the otherwise-idle scalar engine
                    nc.scalar.activation(
                        t2[:, 0:cg], t1[:, :],
                        mybir.ActivationFunctionType.Square,
                        accum_out=acc[:, 1 + NG + g:2 + NG + g])
                else:
                    # trailing groups stay on the DVE: no cross-engine
                    # hop in the tail (t1 * sim == onehot * sim^2)
                    nc.vector.scalar_tensor_tensor(
                        out=t2v[:, 0:cg], in0=t1[:, :], scalar=1.0,
                        in1=ps[:, :], op0=AO.mult, op1=AO.mult,
                        accum_out=acc[:, 1 + NG + g:2 + NG + g])

            # ship the raw per-partition accumulators; host reduces
            nc.sync.dma_start(out[:, :], acc[:, :])

    nc.compile()
    _CACHE["nc"] = nc
    return nc


def _prep_in_maps(emb, lb, segment_queue):
    emb = np.asarray(emb)
    lb = np.asarray(lb)
    q = np.asarray(segment_queue, dtype=np.float32)

    qt = np.zeros((D, CP), np.float32)
    qt[:, :C] = q.T
    # pack [2,128,CP] -> [128, 2*CP]: col 32k+c = QT[128k+p, c]
    qt = np.ascontiguousarray(
        qt.reshape(2, 128, CP).transpose(1, 0, 2).reshape(128, 2 * CP)
        .astype(EMB_NP))

    cls_pat = np.where(np.arange(CP) < C, np.arange(CP), -1)  # [32]

    in_maps = []
    for b in range(B):
        x8 = emb[b].reshape(2, 128, NPX).astype(EMB_NP)
        # pack per DMA tile: [128, 2n] with col k*n+j = x8[k, p, base+j]
        blocks = []
        base = 0
        for n in XTILES:
            blk = x8[:, :, base:base + n]            # [2, 128, n]
            blocks.append(blk.transpose(1, 0, 2).reshape(128, 2 * n))
            base += n
        x0m = np.concatenate([qt, blocks[0]], axis=1)
        xr = np.concatenate(blocks[1:], axis=1)

        lbf = lb[b].reshape(-1).astype(np.float32)
        segs = []
        base = 0
        for n in XTILES:
            cg = n // 4
            seg = lbf[base:base + n].reshape(4, 1, cg)
            segs.append((seg == cls_pat[None, :, None]).reshape(128, cg))
            base += n
        lbl_arr = np.empty((128, LBL_COLS), np.uint8)
        lbl_arr[:, 0:LBB_COLS] = np.concatenate(segs, axis=1)
        lbl_arr[:, LBB_COLS:] = lbf.reshape(128, 128).astype(np.uint8)

        in_maps.append({
            "x0m": np.ascontiguousarray(x0m),
            "xr": np.ascontiguousarray(xr),
            "lbl": np.ascontiguousarray(lbl_arr),
        })
    return in_maps


def _reduce_outputs(results):
    cnt = 0.0
    s1 = 0.0
    s2 = 0.0
    for r in results:
        o = np.asarray(r["out"], dtype=np.float64)
        cnt += o[:, 0].sum()
        s1 += o[:, 1:1 + NG].sum()
        s2 += o[:, 1 + NG:1 + 2 * NG].sum()
    num = cnt - 2.0 * s1 + s2
    return np.float32(num / cnt)


def run_on_cores(inputs, **kwargs):
    """Run the bass kernel on cores 0-7; returns (loss, BassKernelResults).

    The device occasionally reports a transient NRT_EXEC_UNIT_UNRECOVERABLE
    on a run that succeeds on immediate retry; retry a couple of times.
    """
    nc = _build()
    in_maps = _prep_in_maps(**inputs)
    last_err = None
    for _ in range(3):
        try:
            res = bass_utils.run_bass_kernel_spmd(
                nc, in_maps, core_ids=list(range(NCORES)), **kwargs)
            return _reduce_outputs(res.results), res
        except Exception as e:  # transient device wedge -> retry
            last_err = e
    raise last_err


def kernel(emb, lb, segment_queue):
    loss, _ = run_on_cores({"emb": emb, "lb": lb, "segment_queue": segment_queue})
    return loss
